# revision 1
# baseline (speedup 1.0000x reference)
"""Trainium2 Bass kernel for uniform cubic B-spline basis (Cox-de Boor, degree 3).

Uniform knots => all 252 basis functions are shifts of one cardinal cubic C(s)
on [0,4). Row r is zero except columns cstart..cstart+3 (cstart = clip(i-3, 0,
248), i = floor(u), u = (x+pi)/h), holding C(u-cstart-m).

Strategy: batch the output scatter with the custom-ucode `dma_scatter_add`
(out[idxs,:] += in, 2048 rows per SWDGE instruction) instead of one GPSIMD
indirect DMA per 128 rows (994ns fixed overhead x 1024 = the old 1.6ms).
Each row writes one 68-float window at a 256B-aligned address; the window is
computed DENSELY as win[j] = C(w - j), w = (u - cstart) + o, via
    y = |s-2|, C = relu(2-y)^3/6 - (2/3)*relu(1-y)^3   (s outside [0,4] -> 0)
split across DVE and Act. Only full-tensor ops are used on the compute
engines: Bacc's dependency tracker does not model strided-AP overlaps, and
its compile passes will reorder engine instructions whose dependency it
cannot see (found the hard way).

Output layout: DRAM [PC+1, 256] f32, column c of row r at element 256r+4+c.
The +4 shift and the pad row make every window's spill land in pad slots that
only ever receive zero-adds, so concurrent CCE adds never race on real data.
Host slices [:PC, 4:256]. Rows map partition-minor (row r on partition r%128)
so scatter token i == row i. idx = 4*(r%2048... per-prep) + (cstart+4)>>6 and
w are computed on the host in float32 and uploaded; the int16 idx tensor is
16-partition-wrapped and replicated. The runtime pre-zeros ExternalOutput
buffers, so scatter-add acts as scatter-write.
"""
import sys
import types

sys.path.insert(0, "/opt/trn_rl_repo")
sys.path.insert(0, "/root/.axon_site/_ro/trn_rl_repo")

import numpy as np


def _ensure_axon_hooks():
    if "antenv.axon_hooks" in sys.modules:
        return
    try:
        import antenv
    except ImportError:
        return
    m = types.ModuleType("antenv.axon_hooks")
    m._hook = None
    m.set_axon_ntff_profile_hook = lambda h: setattr(m, "_hook", h)
    m.get_axon_ntff_profile_hook = lambda: m._hook
    sys.modules["antenv.axon_hooks"] = m
    antenv.axon_hooks = m
    try:
        from trn_agent_boot.trn_boot import _ntff_profile_via_ctypes

        hook = _ntff_profile_via_ctypes("/opt/axon/libaxon_pjrt.so")
        if hook is not None:
            m.set_axon_ntff_profile_hook(hook)
    except Exception:
        pass


_ensure_axon_hooks()

import concourse.bass as bass
import concourse.bacc as bacc
import concourse.mybir as mybir
from concourse.library_config import mlp as mlp_lib

N = 1_048_576
NCORES = 8
PC = N // NCORES          # 131072 rows per core
P = 128
F = PC // P               # 1024 slots per partition
COLS = 252
OC = 256
NUM_KNOTS = 256

PI = float(np.float32(np.pi))
H = float(np.float32(2.0 * np.pi / (NUM_KNOTS - 1)))
INVH = float(np.float32(1.0) / np.float32(H))
C1 = float(np.float32(PI - 0.5 * H))

AOT = mybir.AluOpType
AFT = mybir.ActivationFunctionType
F32 = mybir.dt.float32
I16 = mybir.dt.int16

FC = 32                   # slots per chunk -> 4096 rows
NCHUNK = F // FC          # 32
PREP = 2048               # tokens per scatter instruction (2 per chunk)
WIN = 68
STEP = 64                 # 256B idx granularity
XDST = 16387
NWBUF = 3


def build_nc():
    nc = bacc.Bacc("TRN2", dynamic_dma_scratch_size=65536)
    w_d = nc.declare_dram_parameter("w", [P, F], F32, isOutput=False)
    ix_d = nc.declare_dram_parameter("idxw", [P, F * 8], I16, isOutput=False)
    out = nc.declare_dram_parameter("out", [PC + 1, OC], F32, isOutput=True)

    w = nc.alloc_sbuf_tensor("w_s", [P, F], F32)
    idxw = nc.alloc_sbuf_tensor("idxw_s", [P, F * 8], I16)
    iotaf = nc.alloc_sbuf_tensor("iotaf", [P, FC * WIN], F32)
    warm_idx = nc.alloc_sbuf_tensor("warm_idx", [P, 1], I16)
    warm_src = nc.alloc_sbuf_tensor("warm_src", [P, WIN], F32)
    bias_m2 = nc.alloc_sbuf_tensor("bias_m2", [P, 1], F32)
    bias_p2 = nc.alloc_sbuf_tensor("bias_p2", [P, 1], F32)
    bias_p1 = nc.alloc_sbuf_tensor("bias_p1", [P, 1], F32)

    sb = [nc.alloc_sbuf_tensor(f"sb{i}", [P, FC * WIN], F32) for i in range(2)]
    zb = [nc.alloc_sbuf_tensor(f"zb{i}", [P, FC * WIN], F32) for i in range(2)]
    ub = [nc.alloc_sbuf_tensor(f"ub{i}", [P, FC * WIN], F32) for i in range(2)]
    zq = nc.alloc_sbuf_tensor("zq", [P, FC * WIN], F32)
    win = [nc.alloc_sbuf_tensor(f"win{i}", [P, FC * WIN], F32) for i in range(NWBUF)]

    with (
        nc.semaphore("insem") as insem,
        nc.semaphore("gsem") as gsem,
        nc.semaphore("sA") as sA,      # DVE s-grid ready
        nc.semaphore("sB") as sB,      # act y/z/u ready
        nc.semaphore("sC") as sC,      # DVE consumed z/u bufs
        nc.semaphore("csem") as csem,  # win content final
        nc.semaphore("prepsem") as prepsem,
        nc.semaphore("dsb0") as dsb0,  # per-win-buffer transfer completion
        nc.semaphore("dsb1") as dsb1,
        nc.semaphore("dsb2") as dsb2,
    ):
        dsb = [dsb0, dsb1, dsb2]
        with nc.Block() as block:

            @block.sync
            def _(s: bass.BassEngine):
                s.dma_start(out=w[:], in_=w_d[:, :]).then_inc(insem, 16)
                s.dma_start(out=idxw[:], in_=ix_d[:, :]).then_inc(insem, 16)

            @block.gpsimd
            def _(g: bass.BassEngine):
                g.iota(warm_idx[:], pattern=[[0, 1]], base=0,
                       channel_multiplier=0)
                g.memset(warm_src[:], 0.0)
                g.memset(bias_m2[:], -2.0)
                g.memset(bias_p2[:], 2.0)
                g.memset(bias_p1[:], 1.0)
                # j-grid 0..67 repeated per slot; f32 ints < 2^24 are exact
                g.iota(iotaf[:], pattern=[[0, FC], [1, WIN]], base=0,
                       channel_multiplier=0,
                       allow_small_or_imprecise_dtypes=True).then_inc(gsem, 1)
                g.wait_ge(gsem, 1)
                g.load_library(mlp_lib)
                # warm the scatter ucode: 16 zero-value descs into the pad
                # row. gen_mode=0 (self-triggered, streams through the ring);
                # the ucode appends the completion-sem descriptors at the end
                # of the stream in both modes, so .then_inc is still a true
                # transfer-drain signal.
                g.dma_scatter_add(
                    out_ap=bass.AP(out, PC * OC, [[STEP, 3], [1, WIN]]),
                    in_ap=warm_src[:, :].rearrange("p (s e) -> p s e", e=WIN),
                    idxs_ap=warm_idx[:, :],
                    num_idxs=16, num_idxs_reg=16,
                    elem_size=WIN, elem_step=STEP,
                ).then_inc(dsb[0], 16)

                g.wait_ge(insem, 32)
                prep_reg = g.to_reg(FC * P)
                for c in range(NCHUNK):
                    g.wait_ge(csem, c + 1)
                    base = c * (FC * P) * OC
                    # one 4096-token instruction per chunk: the Q7 desc-gen
                    # has a big per-instruction fixed cost (it is the
                    # bottleneck engine), so fewer, bigger scatters win.
                    g.dma_scatter_add(
                        out_ap=bass.AP(out, base, [[STEP, XDST], [1, WIN]]),
                        in_ap=win[c % NWBUF][:, :].rearrange(
                            "p (s e) -> p s e", e=WIN),
                        idxs_ap=idxw[
                            :, c * (FC * P // 16): (c + 1) * (FC * P // 16)
                        ],
                        num_idxs=FC * P, num_idxs_reg=prep_reg,
                        elem_size=WIN, elem_step=STEP,
                    ).then_inc(dsb[c % NWBUF], 16)
                for b in range(NWBUF):
                    uses = len([c for c in range(NCHUNK) if c % NWBUF == b])
                    g.wait_ge(dsb[b], 16 * uses + (16 if b == 0 else 0))

            @block.scalar
            def _(a: bass.BassEngine):
                a.wait_ge(gsem, 1)  # bias tensors ready
                for c in range(NCHUNK):
                    a.wait_ge(sA, c + 1)
                    if c >= 2:
                        a.wait_ge(sC, c - 1)  # zb/ub[c%2] free
                    sbc, zbc, ubc = sb[c % 2], zb[c % 2], ub[c % 2]
                    # y = |s - 2| (in place), z = relu(2-y), u = relu(1-y)
                    a.activation(out=sbc[:], in_=sbc[:], func=AFT.Abs,
                                 bias=bias_m2[:, :])
                    a.activation(out=zbc[:], in_=sbc[:], func=AFT.Relu,
                                 bias=bias_p2[:, :], scale=-1.0)
                    last = a.activation(out=ubc[:], in_=sbc[:], func=AFT.Relu,
                                        bias=bias_p1[:, :], scale=-1.0)
                    last.then_inc(sB, 1)

            @block.vector
            def _(ve: bass.BassEngine):
                ve.wait_ge(gsem, 1)
                ve.wait_ge(insem, 32)

                def s_grid(c):
                    # s = w - j over the 68-wide grid
                    ve.tensor_tensor(
                        out=sb[c % 2][:, :].rearrange("p (s e) -> p s e", e=WIN),
                        in0=w[:, c * FC:(c + 1) * FC].unsqueeze(2)
                        .broadcast_to([P, FC, WIN]),
                        in1=iotaf[:, :].rearrange("p (s e) -> p s e", e=WIN),
                        op=AOT.subtract,
                    ).then_inc(sA, 1)

                # software pipeline: compute chunk c+2's s-grid while act
                # processes c+1 and DVE finishes c.
                s_grid(0)
                s_grid(1)
                for c in range(NCHUNK):
                    ve.wait_ge(sB, c + 1)
                    if c + 2 < NCHUNK:
                        s_grid(c + 2)  # sb[c%2] free: act(c) just finished it
                    wb = win[c % NWBUF]
                    b = c % NWBUF
                    need = 16 * (c // NWBUF) + (16 if b == 0 else 0)
                    if need:
                        ve.wait_ge(dsb[b], need)  # win buffer drained
                    zbc, ubc = zb[c % 2], ub[c % 2]
                    # win = relu(2-y)^3/6
                    ve.tensor_tensor(out=zq[:], in0=zbc[:], in1=zbc[:],
                                     op=AOT.mult)
                    ve.scalar_tensor_tensor(out=wb[:], in0=zq[:],
                                            scalar=1.0 / 6.0, in1=zbc[:],
                                            op0=AOT.mult, op1=AOT.mult)
                    # win -= (2/3) relu(1-y)^3
                    ve.tensor_tensor(out=zq[:], in0=ubc[:], in1=ubc[:],
                                     op=AOT.mult)
                    ve.scalar_tensor_tensor(
                        out=zq[:], in0=zq[:], scalar=2.0 / 3.0, in1=ubc[:],
                        op0=AOT.mult, op1=AOT.mult,
                    ).then_inc(sC, 1)  # zb/ub consumed
                    ve.tensor_tensor(out=wb[:], in0=wb[:], in1=zq[:],
                                     op=AOT.subtract).then_inc(csem, 1)

    nc.compile()
    return nc


_CACHED = {}


def make_in_maps(x: np.ndarray) -> list[dict]:
    xs = np.ascontiguousarray(np.asarray(x).reshape(N).astype(np.float32))
    u = (xs + np.float32(PI)) * np.float32(INVH)
    us = (xs + np.float32(C1)) * np.float32(INVH)
    ci = np.rint(us).astype(np.int64)          # == floor(u)
    cst = np.clip(ci - 3, 0, 248)
    dd = u - cst.astype(np.float32)
    e = cst + 4
    k = e >> 6
    o = (e & 63).astype(np.float32)
    wv = dd + o                                # win[j] = C(w - j)
    maps = []
    for c in range(NCORES):
        s = slice(c * PC, (c + 1) * PC)
        r_loc = np.arange(PC, dtype=np.int64)
        idxv = (4 * (r_loc % (FC * P)) + k[s]).astype(np.int16)
        wrapped = np.tile(
            np.ascontiguousarray(idxv.reshape(PC // 16, 16).T), (8, 1)
        )
        maps.append({
            "w": np.ascontiguousarray(wv[s].reshape(F, P).T),
            "idxw": np.ascontiguousarray(wrapped),
        })
    return maps


def kernel(**inputs) -> np.ndarray:
    from concourse.bass_utils import run_bass_kernel_spmd

    x = np.asarray(inputs["x"], dtype=np.float32).reshape(N, 1)
    if "nc" not in _CACHED:
        _CACHED["nc"] = build_nc()
    nc = _CACHED["nc"]
    in_maps = make_in_maps(x)
    res = run_bass_kernel_spmd(nc, in_maps, list(range(NCORES)))
    return np.concatenate(
        [np.ascontiguousarray(r["out"][:PC, 4: 4 + COLS]) for r in res.results],
        axis=0,
    )


if __name__ == "__main__":
    rng = np.random.default_rng(0)
    xs = rng.uniform(-np.pi, np.pi, size=(N, 1)).astype(np.float32)
    o = kernel(x=xs)
    print("out", o.shape, o.dtype, float(np.abs(o).max()))



# revision 8
# speedup vs baseline: 4.0570x; 4.0570x over previous
"""Trainium2 Bass kernel for uniform cubic B-spline basis (Cox-de Boor, degree 3).

Uniform knots => all 252 basis functions are shifts of one cardinal cubic C(s)
on [0,4). Row r is zero except columns cstart..cstart+3 (cstart = clip(i-3, 0,
248), i = floor(u), u = (x+pi)/h), holding C(u-cstart-m).

Strategy (v2, dense + bitwise masks): the original kernel scatter-wrote
68-float windows with the Q7 `dma_scatter_add` ucode; its descriptor
generation (~7.5ns/token * 131072 tokens ~ 1ms) was the bottleneck. Instead,
each output row is materialized DENSELY in SBUF as a 256-wide bf16 slot
(cols 252..255 padding) and streamed out with plain contiguous HWDGE DMA
(67MB/core, DMA engines measured at ~25B/ns each).

Per row: q = cstart>>4 picks one of 16 16-col blocks; w' = (u-cstart) +
(cstart&15) is the eval point of a 20-wide window win[j] = C(w' - j) whose
4-wide support sits in [of16, of16+3] (spans at most blocks q, q+1). The
cubic: y=|s-2|, C = relu(2-y)^3/6 - (2/3)relu(1-y)^3; squares run on Act
(Square(z/sqrt6), Square(t*sqrt(2/3))), so DVE does 3 tensor_tensor ops.

Dense assembly uses BITWISE masking on uint32 lanes (2 bf16 cols per u32
element, halving element count vs bf16 masking): block q' of a slot =
win_u32[0:8] AND mbits[q'], where mbits = 0xFFFFFFFF iff q==q' (host-built,
streamed per chunk). The win[16:20] spill is OR'd into the first 2 u32 words
of block q'+1 (those words are 0 from the AND when the spill is nonzero, so
OR is exact; q'==15 never spills because right-edge clipping keeps of16<=8).
Bitwise ops are DVE-only (NCC_EBIR039: unsupported on the Pool engine), so
all masking runs on DVE; the overlapping spill OR sits behind a
self-semaphore so the strided-AP overlap is an explicit edge (Bacc's
dependency tracker does not model strided-AP overlaps and may reorder
instructions whose dependency it cannot see).

Row r maps partition-major (p = r // F, f = r % F) so each partition's chunk
is one contiguous 32KB DRAM run. Host prep is index arithmetic only
(u/cstart/w'/mask words); host post slices [:, :252] and casts bf16->f32
(values are computed in f32 and rounded once => rel err ~2e-3).
"""
import sys
import types

sys.path.insert(0, "/opt/trn_rl_repo")
sys.path.insert(0, "/root/.axon_site/_ro/trn_rl_repo")

import numpy as np


def _ensure_axon_hooks():
    if "antenv.axon_hooks" in sys.modules:
        return
    try:
        import antenv
    except ImportError:
        return
    m = types.ModuleType("antenv.axon_hooks")
    m._hook = None
    m.set_axon_ntff_profile_hook = lambda h: setattr(m, "_hook", h)
    m.get_axon_ntff_profile_hook = lambda: m._hook
    sys.modules["antenv.axon_hooks"] = m
    antenv.axon_hooks = m
    try:
        from trn_agent_boot.trn_boot import _ntff_profile_via_ctypes

        hook = _ntff_profile_via_ctypes("/opt/axon/libaxon_pjrt.so")
        if hook is not None:
            m.set_axon_ntff_profile_hook(hook)
    except Exception:
        pass


_ensure_axon_hooks()

import concourse.bass as bass
import concourse.bacc as bacc
import concourse.mybir as mybir

N = 1_048_576
NCORES = 8
PC = N // NCORES          # 131072 rows per core
P = 128
F = PC // P               # 1024 slots per partition (partition-major rows)
COLS = 252
OC = 256                  # dense slot width in bf16 cols
OCW = OC // 2             # ... in u32 words
NUM_KNOTS = 256

PI = float(np.float32(np.pi))
H = float(np.float32(2.0 * np.pi / (NUM_KNOTS - 1)))
INVH = float(np.float32(1.0) / np.float32(H))
C1 = float(np.float32(PI - 0.5 * H))

AOT = mybir.AluOpType
AFT = mybir.ActivationFunctionType
F32 = mybir.dt.float32
BF16 = mybir.dt.bfloat16
U32 = mybir.dt.uint32

FC = 64                   # slots per chunk
NCHUNK = F // FC          # 16
WINW = 20                 # window width: 16-col block + 4 spill
NQ = 16                   # 16-col blocks per slot
BW = 8                    # u32 words per block
NQG = 4                   # blocks handled by GPSIMD (12..15); DVE gets 0..11

ISQRT6 = float(np.float32(1.0) / np.float32(np.sqrt(np.float32(6.0))))
SQRT23 = float(np.float32(np.sqrt(np.float32(2.0 / 3.0))))


def build_nc():
    nc = bacc.Bacc("TRN2")
    w_d = nc.declare_dram_parameter("w", [P, F], F32, isOutput=False)
    m_d = nc.declare_dram_parameter("m", [P, F * NQ], U32, isOutput=False)
    out = nc.declare_dram_parameter("out", [PC, OC], BF16, isOutput=True)

    w_s = nc.alloc_sbuf_tensor("w_s", [P, F], F32)
    mb = [nc.alloc_sbuf_tensor(f"mb{i}", [P, FC * NQ], U32) for i in range(2)]
    iota20 = nc.alloc_sbuf_tensor("iota20", [P, FC * WINW], F32)
    bias_m2 = nc.alloc_sbuf_tensor("bias_m2", [P, 1], F32)
    bias_p2 = nc.alloc_sbuf_tensor("bias_p2", [P, 1], F32)
    bias_p1 = nc.alloc_sbuf_tensor("bias_p1", [P, 1], F32)
    bias_0 = nc.alloc_sbuf_tensor("bias_0", [P, 1], F32)

    sb = [nc.alloc_sbuf_tensor(f"sb{i}", [P, FC * WINW], F32) for i in range(2)]
    zb = [nc.alloc_sbuf_tensor(f"zb{i}", [P, FC * WINW], F32) for i in range(2)]
    ub = [nc.alloc_sbuf_tensor(f"ub{i}", [P, FC * WINW], F32) for i in range(2)]
    z2b = [nc.alloc_sbuf_tensor(f"z2b{i}", [P, FC * WINW], F32) for i in range(2)]
    t2b = [nc.alloc_sbuf_tensor(f"t2b{i}", [P, FC * WINW], F32) for i in range(2)]
    winb = [nc.alloc_sbuf_tensor(f"winb{i}", [P, FC * WINW], BF16)
            for i in range(2)]
    spl = [nc.alloc_sbuf_tensor(f"spl{i}", [P, FC * 15 * 2], U32)
           for i in range(2)]
    dense = [nc.alloc_sbuf_tensor(f"dense{i}", [P, FC * OC], BF16)
             for i in range(2)]

    def wview(b):
        # u32 view of the 20 bf16 window cols: words 0..9
        return winb[b][:, :].bitcast(U32).rearrange("p (f j) -> p f j", j=10)

    def dview(b):
        # u32 view of the dense slot: [P, FC, 128 words]
        return dense[b][:, :].bitcast(U32).rearrange("p (f x) -> p f x", x=OCW)

    def dqview(b):
        # u32 view grouped by 16-col block: [P, FC, 16, 8 words]
        return dense[b][:, :].bitcast(U32).rearrange(
            "p (f q j) -> p f q j", q=NQ, j=BW)

    def mview(b):
        return mb[b][:, :].rearrange("p (f q) -> p f q", q=NQ)

    with (
        nc.semaphore("insem") as insem,
        nc.semaphore("gsem") as gsem,
        nc.semaphore("msem") as msem,  # mask-chunk input DMA done
        nc.semaphore("sA") as sA,      # DVE s-grid ready
        nc.semaphore("sB") as sB,      # act y/z/t/z2/t2 ready
        nc.semaphore("sC") as sC,      # DVE win op done (act bufs free too)
        nc.semaphore("sDv") as sDv,    # DVE masked blocks + spill AND done
        nc.semaphore("csem") as csem,  # dense content final (incl. spill OR)
        nc.semaphore("dsem") as dsem,  # out-DMA completion
    ):
        with nc.Block() as block:

            @block.sync
            def _(s: bass.BassEngine):
                s.dma_start(out=w_s[:], in_=w_d[:, :]).then_inc(insem, 16)
                mv_d = m_d[:, :].rearrange("p (f q) -> p f q", q=NQ)
                for c in range(2):
                    s.dma_start(
                        out=mb[c][:],
                        in_=mv_d[:, c * FC:(c + 1) * FC, :],
                    ).then_inc(msem, 16)
                ov = out[:, :].rearrange("(p f) x -> p f x", p=P)
                for c in range(NCHUNK):
                    s.wait_ge(csem, c + 1)
                    s.dma_start(
                        out=ov[:, c * FC:(c + 1) * FC, :],
                        in_=dense[c % 2][:, :].rearrange(
                            "p (f x) -> p f x", x=OC),
                    ).then_inc(dsem, 16)
                    if c + 2 < NCHUNK:
                        # mb[c%2] free once csem(c) fired (all mask readers of
                        # chunk c are ordered before csem via sG -> OR)
                        s.dma_start(
                            out=mb[c % 2][:],
                            in_=mv_d[:, (c + 2) * FC:(c + 3) * FC, :],
                        ).then_inc(msem, 16)
                s.wait_ge(dsem, 16 * NCHUNK)

            @block.gpsimd
            def _(g: bass.BassEngine):
                g.memset(bias_m2[:], -2.0)
                g.memset(bias_p2[:], 2.0)
                g.memset(bias_p1[:], 1.0)
                g.memset(bias_0[:], 0.0)
                # j-grid 0..19 repeated per slot; f32 ints < 2^24 are exact
                g.iota(iota20[:], pattern=[[0, FC], [1, WINW]], base=0,
                       channel_multiplier=0,
                       allow_small_or_imprecise_dtypes=True).then_inc(gsem, 1)

            @block.scalar
            def _(a: bass.BassEngine):
                a.wait_ge(gsem, 1)  # bias tensors ready
                for c in range(NCHUNK):
                    a.wait_ge(sA, c + 1)
                    if c >= 2:
                        a.wait_ge(sC, c - 1)  # bufs[c%2] free
                    sbc, zbc, ubc = sb[c % 2], zb[c % 2], ub[c % 2]
                    z2c, t2c = z2b[c % 2], t2b[c % 2]
                    # y = |s-2| (in place), z = relu(2-y), t = relu(1-y)
                    # z2 = z^2/6 (Square of z/sqrt6), t2 = (2/3) t^2
                    a.activation(out=sbc[:], in_=sbc[:], func=AFT.Abs,
                                 bias=bias_m2[:, :])
                    a.activation(out=zbc[:], in_=sbc[:], func=AFT.Relu,
                                 bias=bias_p2[:, :], scale=-1.0)
                    a.activation(out=ubc[:], in_=sbc[:], func=AFT.Relu,
                                 bias=bias_p1[:, :], scale=-1.0)
                    a.activation(out=z2c[:], in_=zbc[:], func=AFT.Square,
                                 bias=bias_0[:, :], scale=ISQRT6)
                    a.activation(out=t2c[:], in_=ubc[:], func=AFT.Square,
                                 bias=bias_0[:, :],
                                 scale=SQRT23).then_inc(sB, 1)

            @block.vector
            def _(ve: bass.BassEngine):
                ve.wait_ge(gsem, 1)
                ve.wait_ge(insem, 16)

                def s_grid(c):
                    # s = w' - j over the 20-wide grid
                    ve.tensor_tensor(
                        out=sb[c % 2][:, :].rearrange(
                            "p (f j) -> p f j", j=WINW),
                        in0=w_s[:, c * FC:(c + 1) * FC].unsqueeze(2)
                        .broadcast_to([P, FC, WINW]),
                        in1=iota20[:, :].rearrange("p (f j) -> p f j", j=WINW),
                        op=AOT.subtract,
                    ).then_inc(sA, 1)

                s_grid(0)
                s_grid(1)
                for c in range(NCHUNK):
                    b = c % 2
                    ve.wait_ge(sB, c + 1)
                    if c + 2 < NCHUNK:
                        s_grid(c + 2)  # sb[b] free: act(c) finished with it
                    zbc, ubc = zb[b], ub[b]
                    z2c, t2c = z2b[b], t2b[b]
                    # zc = z^3/6 (in place on z2), tc = (2/3) t^3 (in place)
                    ve.tensor_tensor(out=z2c[:], in0=z2c[:], in1=zbc[:],
                                     op=AOT.mult)
                    ve.tensor_tensor(out=t2c[:], in0=t2c[:], in1=ubc[:],
                                     op=AOT.mult)
                    ve.tensor_tensor(out=winb[b][:], in0=z2c[:], in1=t2c[:],
                                     op=AOT.subtract).then_inc(sC, 1)
                    if c >= 2:
                        ve.wait_ge(dsem, 16 * (c - 1))  # dense[b] drained
                    ve.wait_ge(msem, 16 * (c + 1))      # mb[b] loaded
                    wv, dq, mv = wview(b), dqview(b), mview(b)
                    for qi in range(NQ):
                        ve.tensor_tensor(
                            out=dq[:, :, qi, :],
                            in0=wv[:, :, 0:BW],
                            in1=mv[:, :, qi:qi + 1].broadcast_to(
                                [P, FC, BW]),
                            op=AOT.bitwise_and,
                        )
                    # spill words: win u32 words 8:10 masked by the SOURCE
                    # block's mbits (q''=0..14), OR'd into block q''+1 (its
                    # first 2 words are 0 from the AND whenever the spill is
                    # nonzero, so OR is exact)
                    ve.tensor_tensor(
                        out=spl[b][:, :].rearrange(
                            "p (f q j) -> p f q j", q=15, j=2),
                        in0=wv[:, :, 8:10].unsqueeze(2).broadcast_to(
                            [P, FC, 15, 2]),
                        in1=mv[:, :, 0:15].unsqueeze(3).broadcast_to(
                            [P, FC, 15, 2]),
                        op=AOT.bitwise_and,
                    ).then_inc(sDv, 1)
                    # explicit edge before the overlapping OR: Bacc may
                    # reorder same-engine instructions whose strided-AP
                    # dependency it cannot see; the self-sem forces order.
                    ve.wait_ge(sDv, c + 1)
                    dq4 = dqview(b)
                    ve.tensor_tensor(
                        out=dq4[:, :, 1:16, 0:2],
                        in0=dq4[:, :, 1:16, 0:2],
                        in1=spl[b][:, :].rearrange(
                            "p (f q j) -> p f q j", q=15, j=2),
                        op=AOT.bitwise_or,
                    ).then_inc(csem, 1)

    nc.compile()
    return nc


_CACHED = {}


def make_in_maps(x: np.ndarray) -> list[dict]:
    xs = np.ascontiguousarray(np.asarray(x).reshape(N).astype(np.float32))
    u = (xs + np.float32(PI)) * np.float32(INVH)
    us = (xs + np.float32(C1)) * np.float32(INVH)
    ci = np.rint(us).astype(np.int64)          # == floor(u)
    cst = np.clip(ci - 3, 0, 248)
    du = u - cst.astype(np.float32)
    q = (cst >> 4).astype(np.int64)            # 16-col block index, 0..15
    of16 = (cst & 15).astype(np.float32)
    wv = du + of16                             # win[j] = C(wv - j), j in 0..19
    qbits = (q[:, None] == np.arange(NQ)[None, :])
    maps = []
    for c in range(NCORES):
        s = slice(c * PC, (c + 1) * PC)
        mbits = np.where(qbits[s], np.uint32(0xFFFFFFFF), np.uint32(0))
        maps.append({
            "w": np.ascontiguousarray(wv[s].reshape(P, F)),
            "m": np.ascontiguousarray(mbits.astype(np.uint32).reshape(
                P, F * NQ)),
        })
    return maps


def kernel(**inputs) -> np.ndarray:
    from concourse.bass_utils import run_bass_kernel_spmd

    x = np.asarray(inputs["x"], dtype=np.float32).reshape(N, 1)
    if "nc" not in _CACHED:
        _CACHED["nc"] = build_nc()
    nc = _CACHED["nc"]
    in_maps = make_in_maps(x)
    res = run_bass_kernel_spmd(nc, in_maps, list(range(NCORES)))
    return np.concatenate(
        [np.asarray(r["out"])[:, :COLS].astype(np.float32)
         for r in res.results],
        axis=0,
    )


if __name__ == "__main__":
    rng = np.random.default_rng(0)
    xs = rng.uniform(-np.pi, np.pi, size=(N, 1)).astype(np.float32)
    o = kernel(x=xs)
    print("out", o.shape, o.dtype, float(np.abs(o).max()))
